# revision 8
# baseline (speedup 1.0000x reference)
"""Trainium2 Bass kernel for ChamferEigenRatioLoss.

Problem: x, y: [2, 8192, 3] f32 point clouds.
  - idx1[b,i] = argmin_j ||x_i - y_j||^2 ; idx2[b,j] = argmin_i ||x_i - y_j||^2
  - er1/er2: per-point eigen-ratio (lambda_max/lambda_mid of 16-NN covariance)
  - loss = mean over b of 0.5*(mean((er1-er2[idx1])^2) + mean((er2-er1[idx2])^2))

Sharding: 8 cores = 2 batches x 4 row-shards of 2048 query points. Each core
computes, for its query block against the full reference cloud (replicated):
  - scores s_ij = 2 q_i . r_j - |r_j|^2 (argmax_j s == argmin_j dist; the
    row-constant |q_i|^2 is dropped). Computed as THREE fp16 matmuls
    (hi/lo split of both operands, dropping the lo*lo term) accumulated in
    fp32 PSUM — exact to ~2^-22, 2.6x faster than TRN2's native fp32 path.
  - 16-NN selection via DVE max/match_replace (2 rounds, knockout -+2^100),
    mask recovered on ACT as Abs(s * 2^-100) in {1.0, ~1e-28} as bf16
  - neighbor moment sums S1=sum(r), S2=sum(r x r) via maskT @ table bf16
    matmuls on PE (table split hi/lo into 18 cols, summed after)
  - closed-form 3x3 symmetric eigensolver (query-centered covariance) on DVE/ACT
  - argmin indices via DVE max + max_index
Host does only the final O(B*N) index gather + scalar mean.
"""
import os
import sys

sys.path.insert(0, '/opt/trn_rl_repo')

import numpy as np
import ml_dtypes

import concourse.bass as bass
import concourse.tile as tile
from concourse import bacc, mybir
from concourse.bass_utils import run_bass_kernel_spmd
from concourse.masks import make_identity

F32 = mybir.dt.float32
F16 = mybir.dt.float16
BF16 = mybir.dt.bfloat16
U32 = mybir.dt.uint32
AF = mybir.ActivationFunctionType
OP = mybir.AluOpType

B = 2
N = 8192            # points per cloud
SHARDS = 4          # row shards per batch
QBLK = N // SHARDS  # 2048 query points per core
NT = QBLK // 128    # 16 row-tiles per phase
NC_CHUNK = 512      # matmul moving free dim
NCH = N // NC_CHUNK
NJT = N // 128      # 64 j-chunks for moments
KNN = 16
BIG = float(2.0 ** 100)
BIGINV = float(2.0 ** -100)

_KERNEL_CACHE = {}


def _emit_scores(nc, pools, q_sb, ref_sb, t):
    """s_sb [128, 8192] f32 for row-tile t via 3 fp16 matmuls per chunk.

    q_sb: (q_hi, q_lo) each [4, QBLK] f16 augmented
    ref_sb: (r_hi, r_lo) each [4, N] f16 augmented
    """
    psum_s = pools["psum_s"]
    s_sb = pools["s"].tile([128, N], F32, tag="s_tile", name="s_tile")
    qh = q_sb[0][:, t * 128:(t + 1) * 128]
    ql = q_sb[1][:, t * 128:(t + 1) * 128]
    for c2 in range(NCH // 2):
        ps = psum_s.tile([128, 2 * NC_CHUNK], F32, tag="ps_s", name="ps_s")
        for u in range(2):
            c = 2 * c2 + u
            rh = ref_sb[0][:, c * NC_CHUNK:(c + 1) * NC_CHUNK]
            rl = ref_sb[1][:, c * NC_CHUNK:(c + 1) * NC_CHUNK]
            out = ps[:, u * NC_CHUNK:(u + 1) * NC_CHUNK]
            nc.tensor.matmul(out, qh, rh, start=True, stop=False)
            nc.tensor.matmul(out, qh, rl, start=False, stop=False)
            nc.tensor.matmul(out, ql, rh, start=False, stop=True)
        nc.scalar.copy(s_sb[:, c2 * 2 * NC_CHUNK:(c2 + 1) * 2 * NC_CHUNK], ps[:])
    return s_sb


def _emit_select(nc, pools, s_sb):
    """Top-16 marking on DVE + bf16 mask on ACT. Returns mask tile."""
    m8p = pools["m8"]
    m8a = m8p.tile([128, 8], F32, tag="m8", name="m8a")
    nc.vector.max(out=m8a[:], in_=s_sb[:])
    nc.vector.match_replace(out=s_sb[:], in_to_replace=m8a[:],
                            in_values=s_sb[:], imm_value=-BIG)
    m8b = m8p.tile([128, 8], F32, tag="m8", name="m8b")
    nc.vector.max(out=m8b[:], in_=s_sb[:])
    nc.vector.match_replace(out=s_sb[:], in_to_replace=m8b[:],
                            in_values=s_sb[:], imm_value=BIG)
    mask = pools["mask"].tile([128, N], BF16, tag="mask", name="mask")
    nc.scalar.activation(out=mask[:], in_=s_sb[:], func=AF.Abs, scale=BIGINV)
    return mask


def _emit_transmom(nc, pools, mask, tab_sb, identity, moments_sb, t):
    """Transpose mask 128x128 blocks on PE, then bf16 moment matmuls."""
    psum_t = pools["psum_t"]
    psum_m = pools["psum_m"]
    mtp = pools["mt"]
    pm = psum_m.tile([128, 27], F32, tag="pmom", name="pmom")
    for g in range(NJT // 4):
        pt = psum_t.tile([128, 4, 128], BF16, tag="pt", name="pt")
        for u in range(4):
            c2 = 4 * g + u
            nc.tensor.transpose(pt[:, u, :], mask[:, c2 * 128:(c2 + 1) * 128],
                                identity)
        mt = mtp.tile([128, 4, 128], BF16, tag="mt", name="mt")
        nc.scalar.copy(mt[:], pt[:])
        for u in range(4):
            c2 = 4 * g + u
            nc.tensor.matmul(
                pm[:], mt[:, u, :], tab_sb[:, c2, :],
                start=(c2 == 0), stop=(c2 == NJT - 1),
            )
    nc.scalar.copy(moments_sb[:, t, :], pm[:])


def _emit_knn_phase(nc, pools, q_sb, ref_sb, tab_sb, identity, moments_sb):
    """Software-pipelined at emission level: PE order is
    scores(0), scores(1), transmom(0), scores(2), transmom(1), ...
    so the PE never stalls waiting for tile t's DVE selection."""
    pending = None  # (mask, t)
    for t in range(NT):
        s_sb = _emit_scores(nc, pools, q_sb, ref_sb, t)
        if pending is not None:
            _emit_transmom(nc, pools, pending[0], tab_sb, identity,
                           moments_sb, pending[1])
        mask = _emit_select(nc, pools, s_sb)
        pending = (mask, t)
    _emit_transmom(nc, pools, pending[0], tab_sb, identity, moments_sb,
                   pending[1])


def _emit_idx_phase(nc, pools, q_sb, ref_sb, idx_sb, col0):
    for t in range(NT):
        s_sb = _emit_scores(nc, pools, q_sb, ref_sb, t)
        m8 = pools["m8"].tile([128, 8], F32, tag="m8", name="m8i")
        nc.vector.max(out=m8[:], in_=s_sb[:])
        i8 = pools["i8"].tile([128, 8], U32, tag="i8", name="i8")
        nc.vector.max_index(i8[:], m8[:], s_sb[:])
        nc.vector.tensor_copy(out=idx_sb[:, col0 + t:col0 + t + 1], in_=i8[:, 0:1])


def _emit_eigen(nc, pools, moments_sb, cen_sb, er_out_ap):
    """Closed-form lambda_max/lambda_mid of the 16-NN covariance.

    moments_sb: [128, NT, 27] f32 — cols 0-8/9-17/18-26 are hi/mid/lo-table
                sums of (S1 xyz | S2 xx xy xz yy yz zz)
    cen_sb:     [128, NT, 3] f32 query coords (centering)
    er_out_ap:  [128, NT] f32 destination

    The covariance uses query-centering with a compensated (TwoProd) product
    for the one catastrophic cancellation S2_ab - q_a*S1_b.
    """
    sc = pools["eig"]
    K = float(KNN)

    def T(tag):
        return sc.tile([128, NT], F32, tag=tag, name=f"eig_{tag}")

    v = nc.vector
    # S = hi + mid + lo
    S1 = []
    for a in range(3):
        s1a = T(f"s1{a}")
        v.tensor_add(s1a, moments_sb[:, :, a], moments_sb[:, :, 9 + a])
        v.tensor_add(s1a, s1a, moments_sb[:, :, 18 + a])
        S1.append(s1a)
    S2 = {}
    for i, (a, b) in enumerate([(0, 0), (0, 1), (0, 2), (1, 1), (1, 2), (2, 2)]):
        s2 = T(f"s2{a}{b}")
        v.tensor_add(s2, moments_sb[:, :, 3 + i], moments_sb[:, :, 12 + i])
        v.tensor_add(s2, s2, moments_sb[:, :, 21 + i])
        S2[(a, b)] = s2
    q = [cen_sb[:, :, a] for a in range(3)]

    # H_a = S1_a - K q_a (small, ~K * local radius); mu'_a = H_a / K
    h = [T(f"h{b}") for b in range(3)]
    mu = [T(f"mu{b}") for b in range(3)]
    for a in range(3):
        v.scalar_tensor_tensor(h[a], q[a], -K, S1[a], op0=OP.mult, op1=OP.add)
        v.tensor_scalar_mul(mu[a], h[a], 1.0 / K)

    # Dekker splits of q_a and S1_b (12+12 mantissa bits) for TwoProd
    def split(val, nm):
        c = T(f"sp_c")
        hi_ = T(f"{nm}_hi")
        lo_ = T(f"{nm}_lo")
        v.tensor_scalar_mul(c, val, 4097.0)
        v.tensor_sub(hi_, c, val)        # c - v = v*4096
        v.tensor_sub(hi_, c, hi_)        # hi = c - (c - v)
        v.tensor_sub(lo_, val, hi_)
        return hi_, lo_

    qs = [split(q[a], f"q{a}") for a in range(3)]
    ss = [split(S1[a], f"s{a}") for a in range(3)]

    # cov_ab = (S2_ab - TwoProd(q_a, S1_b) - q_b H_a)/K - mu_a mu_b
    cov = {}
    t1 = T("t1")
    t2 = T("t2")
    for (a, b) in [(0, 0), (0, 1), (0, 2), (1, 1), (1, 2), (2, 2)]:
        cab = T(f"c{a}{b}")
        p_ = T("tp_p")
        e_ = T("tp_e")
        v.tensor_mul(p_, q[a], S1[b])
        v.tensor_mul(e_, qs[a][0], ss[b][0])
        v.tensor_sub(e_, e_, p_)
        v.tensor_mul(t1, qs[a][0], ss[b][1])
        v.tensor_add(e_, e_, t1)
        v.tensor_mul(t1, qs[a][1], ss[b][0])
        v.tensor_add(e_, e_, t1)
        v.tensor_mul(t1, qs[a][1], ss[b][1])
        v.tensor_add(e_, e_, t1)          # e = exact(q_a*S1_b) - p
        v.tensor_sub(cab, S2[(a, b)], p_)
        v.tensor_sub(cab, cab, e_)
        v.tensor_mul(t1, q[b], h[a])
        v.tensor_sub(cab, cab, t1)        # D_ab
        v.tensor_mul(t1, mu[a], mu[b])
        v.scalar_tensor_tensor(cab, cab, 1.0 / K, t1, op0=OP.mult, op1=OP.subtract)
        cov[(a, b)] = cab
    c00, c01, c02 = cov[(0, 0)], cov[(0, 1)], cov[(0, 2)]
    c11, c12, c22 = cov[(1, 1)], cov[(1, 2)], cov[(2, 2)]

    qq = T("qq")
    v.tensor_add(t1, c00, c11)
    v.tensor_add(t1, t1, c22)
    v.tensor_scalar_mul(qq, t1, 1.0 / 3.0)
    b00, b11, b22 = T("b00"), T("b11"), T("b22")
    v.tensor_sub(b00, c00, qq)
    v.tensor_sub(b11, c11, qq)
    v.tensor_sub(b22, c22, qq)
    # p2 = b00^2+b11^2+b22^2 + 2(c01^2+c02^2+c12^2)
    p2 = T("p2")
    v.tensor_mul(p2, b00, b00)
    v.tensor_mul(t1, b11, b11)
    v.tensor_add(p2, p2, t1)
    v.tensor_mul(t1, b22, b22)
    v.tensor_add(p2, p2, t1)
    v.tensor_mul(t1, c01, c01)
    v.tensor_mul(t2, c02, c02)
    v.tensor_add(t1, t1, t2)
    v.tensor_mul(t2, c12, c12)
    v.tensor_add(t1, t1, t2)
    v.scalar_tensor_tensor(p2, t1, 2.0, p2, op0=OP.mult, op1=OP.add)
    p = T("p")
    nc.scalar.activation(out=p, in_=p2, func=AF.Sqrt, scale=1.0 / 6.0)
    pinv = T("pinv")
    v.tensor_scalar_max(t1, p, 1e-30)
    v.reciprocal(pinv, t1)
    # det(C - qq I)
    det = T("det")
    v.tensor_mul(t1, b11, b22)
    v.tensor_mul(t2, c12, c12)
    v.tensor_sub(t1, t1, t2)
    v.tensor_mul(det, b00, t1)
    v.tensor_mul(t1, c01, b22)
    v.tensor_mul(t2, c12, c02)
    v.tensor_sub(t1, t1, t2)
    v.tensor_mul(t1, c01, t1)
    v.tensor_sub(det, det, t1)
    v.tensor_mul(t1, c01, c12)
    v.tensor_mul(t2, b11, c02)
    v.tensor_sub(t1, t1, t2)
    v.tensor_mul(t1, c02, t1)
    v.tensor_add(det, det, t1)
    # r = clamp(det/(2 p^3), [-1, 1])
    r = T("r")
    v.tensor_mul(t1, pinv, pinv)
    v.tensor_mul(t1, t1, pinv)
    v.scalar_tensor_tensor(r, det, 0.5, t1, op0=OP.mult, op1=OP.mult)
    v.tensor_scalar_min(r, r, 1.0)
    v.tensor_scalar_max(r, r, -1.0)
    # at = arctan(r / sqrt(1 - r^2)) ; acos(r) = pi/2 - at ; phi = acos/3
    u = T("u")
    v.tensor_mul(t1, r, r)
    v.tensor_scalar(u, t1, -1.0, 1.0, op0=OP.mult, op1=OP.add)
    v.tensor_scalar_max(u, u, 0.0)
    s_ = T("s_")
    nc.scalar.activation(out=s_, in_=u, func=AF.Sqrt)
    v.tensor_scalar_max(t1, s_, 1e-20)
    v.reciprocal(t2, t1)
    v.tensor_mul(t1, r, t2)
    at = T("at")
    nc.scalar.activation(out=at, in_=t1, func=AF.Arctan)
    # cos(phi) = sin(pi/3 + at/3) ; cos(phi + 2pi/3) = sin(at/3 - pi/3)
    cphi = T("cphi")
    nc.scalar.activation(out=cphi, in_=at, func=AF.Sin, scale=1.0 / 3.0,
                         bias=float(np.pi / 3.0))
    cphi3 = T("cphi3")
    nc.scalar.activation(out=cphi3, in_=at, func=AF.Sin, scale=1.0 / 3.0,
                         bias=float(-np.pi / 3.0))
    e1, e3 = T("e1"), T("e3")
    v.tensor_mul(t1, p, cphi)
    v.scalar_tensor_tensor(e1, t1, 2.0, qq, op0=OP.mult, op1=OP.add)
    v.tensor_mul(t1, p, cphi3)
    v.scalar_tensor_tensor(e3, t1, 2.0, qq, op0=OP.mult, op1=OP.add)
    v.scalar_tensor_tensor(t2, qq, 3.0, e1, op0=OP.mult, op1=OP.subtract)
    v.tensor_sub(t2, t2, e3)
    v.tensor_scalar_max(t2, t2, 1e-30)
    v.reciprocal(t1, t2)
    v.tensor_mul(er_out_ap, e1, t1)


def _register_const(nc, value):
    t = nc.alloc_sbuf_tensor(f"const-f32-{value}", [128, 1], F32)
    nc.gpsimd.memset(t.ap(), value)
    nc.const_aps.aps[(F32, float(value))] = t.ap()


def build_kernel():
    nc = bacc.Bacc(None, target_bir_lowering=False)
    _register_const(nc, float(np.pi / 3.0))
    _register_const(nc, float(-np.pi / 3.0))
    nc.all_engine_barrier()
    qxh = nc.dram_tensor("qxh", [4, QBLK], F16, kind="ExternalInput")
    qxl = nc.dram_tensor("qxl", [4, QBLK], F16, kind="ExternalInput")
    qyh = nc.dram_tensor("qyh", [4, QBLK], F16, kind="ExternalInput")
    qyl = nc.dram_tensor("qyl", [4, QBLK], F16, kind="ExternalInput")
    rxh = nc.dram_tensor("rxh", [4, N], F16, kind="ExternalInput")
    rxl = nc.dram_tensor("rxl", [4, N], F16, kind="ExternalInput")
    ryh = nc.dram_tensor("ryh", [4, N], F16, kind="ExternalInput")
    ryl = nc.dram_tensor("ryl", [4, N], F16, kind="ExternalInput")
    tx = nc.dram_tensor("tx", [128, NJT, 27], BF16, kind="ExternalInput")
    ty = nc.dram_tensor("ty", [128, NJT, 27], BF16, kind="ExternalInput")
    cx = nc.dram_tensor("cx", [128, NT, 3], F32, kind="ExternalInput")
    cy = nc.dram_tensor("cy", [128, NT, 3], F32, kind="ExternalInput")
    er_out = nc.dram_tensor("er_out", [128, 2 * NT], F32, kind="ExternalOutput")
    idx_out = nc.dram_tensor("idx_out", [128, 2 * NT], U32, kind="ExternalOutput")

    from contextlib import ExitStack
    with tile.TileContext(nc) as tc, ExitStack() as ctx:
        pools = {}
        pools["singles"] = ctx.enter_context(tc.tile_pool(name="singles", bufs=1))
        pools["ref"] = ctx.enter_context(tc.tile_pool(name="ref", bufs=4))
        pools["s"] = ctx.enter_context(tc.tile_pool(name="s", bufs=2))
        pools["mask"] = ctx.enter_context(tc.tile_pool(name="mask", bufs=2))
        pools["mt"] = ctx.enter_context(tc.tile_pool(name="mt", bufs=4))
        pools["m8"] = ctx.enter_context(tc.tile_pool(name="m8", bufs=4))
        pools["i8"] = ctx.enter_context(tc.tile_pool(name="i8", bufs=4))
        pools["eig"] = ctx.enter_context(tc.tile_pool(name="eig", bufs=1))
        pools["mom"] = ctx.enter_context(tc.tile_pool(name="mom", bufs=2))
        pools["psum_s"] = ctx.enter_context(
            tc.tile_pool(name="psum_s", bufs=2, space="PSUM"))
        pools["psum_t"] = ctx.enter_context(
            tc.tile_pool(name="psum_t", bufs=2, space="PSUM"))
        pools["psum_m"] = ctx.enter_context(
            tc.tile_pool(name="psum_m", bufs=2, space="PSUM"))

        singles = pools["singles"]
        identity = singles.tile([128, 128], BF16)
        make_identity(nc, identity)

        qxh_sb = singles.tile([4, QBLK], F16)
        nc.sync.dma_start(qxh_sb[:], qxh[:])
        qxl_sb = singles.tile([4, QBLK], F16)
        nc.sync.dma_start(qxl_sb[:], qxl[:])
        qyh_sb = singles.tile([4, QBLK], F16)
        nc.sync.dma_start(qyh_sb[:], qyh[:])
        qyl_sb = singles.tile([4, QBLK], F16)
        nc.sync.dma_start(qyl_sb[:], qyl[:])
        cx_sb = singles.tile([128, NT, 3], F32)
        nc.sync.dma_start(cx_sb[:], cx[:])
        cy_sb = singles.tile([128, NT, 3], F32)
        nc.sync.dma_start(cy_sb[:], cy[:])
        tx_sb = singles.tile([128, NJT, 27], BF16)
        nc.sync.dma_start(tx_sb[:], tx[:])
        ty_sb = singles.tile([128, NJT, 27], BF16)
        nc.sync.dma_start(ty_sb[:], ty[:])

        er_sb = singles.tile([128, 2 * NT], F32)
        idx_sb = singles.tile([128, 2 * NT], U32)

        momx = pools["mom"].tile([128, NT, 27], F32, tag="mom", name="momx")
        momy = pools["mom"].tile([128, NT, 27], F32, tag="mom", name="momy")

        rxh_sb = pools["ref"].tile([4, N], F16, tag="ref", name="rxh_sb")
        nc.sync.dma_start(rxh_sb[:], rxh[:])
        rxl_sb = pools["ref"].tile([4, N], F16, tag="ref", name="rxl_sb")
        nc.sync.dma_start(rxl_sb[:], rxl[:])
        ryh_sb = pools["ref"].tile([4, N], F16, tag="ref", name="ryh_sb")
        nc.sync.dma_start(ryh_sb[:], ryh[:])
        ryl_sb = pools["ref"].tile([4, N], F16, tag="ref", name="ryl_sb")
        nc.sync.dma_start(ryl_sb[:], ryl[:])

        # phases ordered by reference-cloud residency
        qx_sb = (qxh_sb, qxl_sb)
        qy_sb = (qyh_sb, qyl_sb)
        rx_sb = (rxh_sb, rxl_sb)
        ry_sb = (ryh_sb, ryl_sb)
        _emit_knn_phase(nc, pools, qx_sb, rx_sb, tx_sb, identity, momx)
        _emit_idx_phase(nc, pools, qy_sb, rx_sb, idx_sb, NT)   # idx2 block
        _emit_idx_phase(nc, pools, qx_sb, ry_sb, idx_sb, 0)    # idx1 block
        _emit_knn_phase(nc, pools, qy_sb, ry_sb, ty_sb, identity, momy)

        _emit_eigen(nc, pools, momx, cx_sb, er_sb[:, 0:NT])
        _emit_eigen(nc, pools, momy, cy_sb, er_sb[:, NT:2 * NT])

        nc.sync.dma_start(er_out[:], er_sb[:])
        nc.sync.dma_start(idx_out[:], idx_sb[:])

    nc.finalize()
    return nc


def _split16(v64):
    """fp64 array -> (fp16 hi, fp16 lo) with hi+lo ~ v to ~2^-22 rel."""
    hi = v64.astype(np.float16)
    lo = (v64 - hi.astype(np.float64)).astype(np.float16)
    return hi, lo


def _splitbf(v64):
    """fp64 array -> (bf16 hi, bf16 lo) with hi+lo ~ v to ~2^-17 rel."""
    hi = v64.astype(ml_dtypes.bfloat16)
    lo = (v64 - hi.astype(np.float64)).astype(ml_dtypes.bfloat16)
    return hi, lo


def _prep_core_inputs(xb, yb, s):
    """Per-core input dict. xb, yb: [N, 3] f32 clouds of this batch; s: shard."""
    def aug_query(pts):
        blk = pts[s * QBLK:(s + 1) * QBLK].astype(np.float64)  # [QBLK, 3]
        oh = np.zeros((4, QBLK), np.float16)
        ol = np.zeros((4, QBLK), np.float16)
        hi, lo = _split16(blk.T)
        oh[0:3] = hi
        ol[0:3] = lo
        oh[3] = 1.0
        return oh, ol

    def aug_ref(pts):
        p = pts.astype(np.float64)
        oh = np.zeros((4, N), np.float16)
        ol = np.zeros((4, N), np.float16)
        hi, lo = _split16(2.0 * p.T)
        oh[0:3] = hi
        ol[0:3] = lo
        n = np.sum(p * p, axis=1)
        nh, nl = _split16(-n)
        oh[3] = nh
        ol[3] = nl
        return oh, ol

    def mom_table(pts):
        p = pts.astype(np.float64).reshape(NJT, 128, 3).transpose(1, 0, 2)
        vals = np.empty((128, NJT, 9), np.float64)
        vals[:, :, 0:3] = p
        vals[:, :, 3] = p[:, :, 0] * p[:, :, 0]
        vals[:, :, 4] = p[:, :, 0] * p[:, :, 1]
        vals[:, :, 5] = p[:, :, 0] * p[:, :, 2]
        vals[:, :, 6] = p[:, :, 1] * p[:, :, 1]
        vals[:, :, 7] = p[:, :, 1] * p[:, :, 2]
        vals[:, :, 8] = p[:, :, 2] * p[:, :, 2]
        hi = vals.astype(ml_dtypes.bfloat16)
        rem = vals - hi.astype(np.float64)
        mid = rem.astype(ml_dtypes.bfloat16)
        lo = (rem - mid.astype(np.float64)).astype(ml_dtypes.bfloat16)
        out = np.empty((128, NJT, 27), ml_dtypes.bfloat16)
        out[:, :, 0:9] = hi
        out[:, :, 9:18] = mid
        out[:, :, 18:27] = lo
        return out

    def centers(pts):
        blk = pts[s * QBLK:(s + 1) * QBLK]
        return np.ascontiguousarray(
            blk.reshape(NT, 128, 3).transpose(1, 0, 2)).astype(np.float32)

    qxh_, qxl_ = aug_query(xb)
    qyh_, qyl_ = aug_query(yb)
    rxh_, rxl_ = aug_ref(xb)
    ryh_, ryl_ = aug_ref(yb)
    return {
        "qxh": qxh_, "qxl": qxl_, "qyh": qyh_, "qyl": qyl_,
        "rxh": rxh_, "rxl": rxl_, "ryh": ryh_, "ryl": ryl_,
        "tx": mom_table(xb), "ty": mom_table(yb),
        "cx": centers(xb), "cy": centers(yb),
    }


def run_device(x, y, trace=False, trace_kwargs=None):
    """Run the 8-core SPMD kernel; returns (er1, er2, idx1, idx2, results)."""
    if "nc" not in _KERNEL_CACHE:
        _KERNEL_CACHE["nc"] = build_kernel()
    nc = _KERNEL_CACHE["nc"]
    in_maps = []
    for core in range(8):
        b, s = divmod(core, SHARDS)
        in_maps.append(_prep_core_inputs(x[b], y[b], s))
    kw = dict(trace_kwargs or {})
    res = run_bass_kernel_spmd(nc, in_maps, core_ids=list(range(8)),
                               trace=trace, **kw)
    er1 = np.empty((B, N), np.float32)
    er2 = np.empty((B, N), np.float32)
    idx1 = np.empty((B, N), np.int64)
    idx2 = np.empty((B, N), np.int64)
    for core in range(8):
        b, s = divmod(core, SHARDS)
        r = res.results[core]
        er = r["er_out"]                       # [128, 2*NT]
        ix = r["idx_out"].astype(np.int64)     # [128, 2*NT]
        base = s * QBLK
        for t in range(NT):
            sl = slice(base + t * 128, base + (t + 1) * 128)
            er1[b, sl] = er[:, t]
            er2[b, sl] = er[:, NT + t]
            idx1[b, sl] = ix[:, t]
            idx2[b, sl] = ix[:, NT + t]
    return er1, er2, idx1, idx2, res


def kernel(x, y):
    x = np.asarray(x, dtype=np.float32)
    y = np.asarray(y, dtype=np.float32)
    er1, er2, idx1, idx2, _ = run_device(x, y)
    dists = []
    for b in range(B):
        corr_er1 = er2[b][idx1[b]]
        corr_er2 = er1[b][idx2[b]]
        d1 = np.mean((er1[b] - corr_er1) ** 2, dtype=np.float64)
        d2 = np.mean((er2[b] - corr_er2) ** 2, dtype=np.float64)
        dists.append(0.5 * (d1 + d2))
    return np.float32(np.mean(dists))
